# revision 7
# baseline (speedup 1.0000x reference)
"""Trainium2 Bass kernel for nn_Euclidean (retrieval_knn).

Computes out[b, c] = -mean_f (x[b, f] - w[c, f])^2 for x [16384, 2048] f32,
w [1000, 2048] f32, via the algebraic expansion

    out = (2/F) * (x @ w.T  -  ||w_c||^2/2)  -  ||x_b||^2 / F

Sharding: data-parallel over the batch dim across 8 NeuronCores; w replicated.

Schedule (v4): lag-3 software pipeline; phase G round r issues
[x_load(r+3), gemm(r), x_prep(r+3)] so the PE stays HAM-warm.

  - w loads split across BOTH HWDGE rings (w0-3 scalar, w4-7 sync); x
    loads queue behind w4-7 on the sync ring FIFO so w gets full DMA
    bandwidth first.  All issues precede any compute instruction.
  - ||w_c||^2 comes from the PE: 8 DoubleRow matmuls square-contract the
    already-transposed fp8 wT chunks (diag of wT.T@wT in an idle gemm
    PSUM buffer), then DVE identity-mask multiply + ScalarE accumulate
    extract the diagonal.  This keeps the 2us-per-tile Square
    activations off the front-critical ScalarE queue.
  - w2 column -> row via DRAM round-trip on the GpSimd SWDGE ring
    (never blocks HWDGE x loads); -w2/2 lands in row 0 of the bias-rider
    wT plane.
  - gemm output stores ride GpSimd SWDGE too.

Per-x-tile prep: HWDGE DMA fp32 -> DVE bf16 cast -> ScalarE Square
activation with fp32 accum giving ||x_b||^2/F (ScalarE has slack in
phase G) -> TensorE bf16 transposes (2 groups of 8, one [128,1024] PSUM
tile each) -> 1 ScalarE + 1 DVE evac -> fp8e4 SBUF.

GEMM: 9 DoubleRow fp8 matmuls per 512-col half accumulate into one
[128,1024] PSUM tile (two banks).  The first plane-pair is a constant:
row 0 of its xT plane is all-ones and row 0 of its wT plane holds
-||w_c||^2/2 (fp8), so the bias term rides the same DR pipeline with no
weight-mode switch.  lhsT/rhs are 3D APs [128 ki, 2 plane, free] whose
planes are adjacent 128-k chunks (plane stride %16==0 satisfies
s3_lw_dual_fp8_restrictions).  One ScalarE Identity evacuates both PSUM
banks with scale=2/F and per-partition bias -||x_b||^2/F.

Walrus encodes at most one semaphore wait per LDWEIGHTS/MM struct: dummy
standalone LDWEIGHTS absorb cross-engine waits ahead of each transpose
group (add_dep_helper keeps them ordered), and _legalize_waits splits any
remaining multi-wait instructions.
"""

import math
import os
import sys

import numpy as np

if "/opt/trn_rl_repo" not in sys.path:
    sys.path.insert(0, "/opt/trn_rl_repo")

N_CORES = 8
B_TOTAL = 16384
F = 2048
C = 1000

_cache = {}
LAST_RESULTS = None


def _legalize_waits(nc):
    """Walrus encodes at most ONE sync-wait per instruction struct, but Tile's
    sem assignment freely attaches several. Split: hoist all but the last wait
    onto standalone EventSemaphore instructions (pure sem-op carriers) placed
    immediately before the over-limit instruction on the same engine queue."""
    import bass_rust
    import concourse.mybir as mybir

    n = 0
    for f in nc.m.functions:
        for bb in f.blocks:
            newlist = []
            for inst in bb.instructions:
                si = inst.sync_info
                if si is not None and len(si.on_wait) > 1:
                    waits = list(si.on_wait)
                    for w in waits[:-1]:
                        ev = mybir.InstEventSemaphore(
                            name=f"waitsplit_{n}", ins=[], outs=[]
                        )
                        ev.engine = inst.engine
                        ev.sync_info = bass_rust.SyncInfo(on_wait=[w], on_update=[])
                        newlist.append(ev)
                        n += 1
                    inst.sync_info = bass_rust.SyncInfo(
                        on_wait=[waits[-1]], on_update=list(si.on_update)
                    )
                newlist.append(inst)
            bb.instructions = newlist
    return n


def _build():
    import concourse.bass as bass
    import concourse.mybir as mybir
    from bass_rust import add_dep_helper
    from concourse.masks import make_identity
    from concourse.tile import TileContext

    P = 128
    KT = F // P                 # 16 contraction chunks of 128
    KD = KT // 2                # 8 DoubleRow plane-pairs of 256
    B = B_TOTAL // N_CORES      # 2048 batch rows per core
    BT = B // P                 # 16 batch chunks
    CP = 1024                   # padded class dim
    CT = CP // P                # 8 class chunks
    KG = 8                      # k-chunks per PSUM transpose group
    LAG = 3                     # x tiles prepped ahead of their gemm
    f8 = mybir.dt.float8e4
    bdt = mybir.dt.bfloat16
    fdt = mybir.dt.float32
    AF = mybir.ActivationFunctionType
    ALU = mybir.AluOpType
    DR = mybir.MatmulPerfMode.DoubleRow

    nc = bass.Bass()
    x = nc.dram_tensor("x", [B, F], fdt, kind="ExternalInput")
    w = nc.dram_tensor("w", [C, F], fdt, kind="ExternalInput")
    out = nc.dram_tensor("out", [B, C], fdt, kind="ExternalOutput")

    with TileContext(nc) as tc:
        with (
            tc.tile_pool(name="consts", bufs=1) as constp,
            tc.tile_pool(name="wstage", bufs=3) as wp,
            tc.tile_pool(name="xstage", bufs=3) as xp,
            tc.tile_pool(name="evac", bufs=3) as ep,
            tc.tile_pool(name="dram", bufs=1, space="DRAM") as dp,
            tc.tile_pool(name="psum", bufs=2, space="PSUM") as pp,
        ):
            # ---- DMA issues first: w on both rings, x behind w on sync ----
            w_f32s = []
            for j in range(CT):
                c0 = j * P
                csz = min(P, C - c0)
                w_f32 = wp.tile([P, F], fdt, tag="w_f32", bufs=8,
                                name=f"w_f32_{j}")
                eng = nc.scalar if j < CT // 2 else nc.sync
                eng.dma_start(out=w_f32[:csz, :], in_=w[c0 : c0 + csz, :])
                w_f32s.append(w_f32)

            def x_load(i):
                x_f32 = xp.tile([P, F], fdt, tag="x_f32", bufs=3,
                                name=f"x_f32_{i}")
                nc.sync.dma_start(out=x_f32[:, :], in_=x[i * P : (i + 1) * P, :])
                return x_f32

            xfs = {}
            for j in range(LAG):
                xfs[j] = x_load(j)

            # ---- constants (emitted after the issues; run during the DMA) --
            ident = constp.tile([P, P], bdt)
            make_identity(nc, ident[:, :])
            pwarm = pp.tile([P, P], bdt, tag="pst", bufs=2)
            nc.tensor.transpose(pwarm[:, :], ident[:, :], ident[:, :])

            # preload both ACT table sets (Square + Identity) off-path
            tw0 = wp.tile([1, 1], fdt, tag="actwarm0")
            tw1 = wp.tile([1, 1], fdt, tag="actwarm1")
            nc.vector.memset(tw0[:, :], 0.0)
            nc.scalar.activation(tw1[:, :], tw0[:, :], AF.Square)
            nc.scalar.activation(tw1[:, :], tw0[:, :], AF.Identity)

            # wT planes 0..15 hold w^T fp8; plane pair 16/17 is the bias
            # rider: row 0 of plane 16 carries -||w_c||^2/2, rest zero.
            wT = constp.tile([P, KT + 2, CP], f8)
            xT1 = constp.tile([P, 2, P], f8)      # ones@row0 bias-rider lhsT
            nc.vector.memset(wT[:, KT : KT + 2, :], 0.0)
            nc.vector.memset(xT1[:, :, :], 0.0)
            nc.vector.memset(xT1[0:1, 0:1, :], 1.0)
            w2row = constp.tile([1, CP], fdt)
            w2d = dp.tile([CP, 1], fdt)
            xTs = [
                constp.tile([P, KT, P], f8, name=f"xT_{i}") for i in range(BT)
            ]
            negx2s = [
                constp.tile([P, 1], fdt, name=f"negx2_{i}") for i in range(BT)
            ]

            dum_pool = {"prev": None}

            def transpose_evac(bftile, put_evac):
                """16 bf16 transposes in 2 [128,1024] PSUM groups + evacs."""
                dums = [nc.tensor.ldweights(bftile[:, 0:P])]
                if dum_pool["prev"] is not None:
                    dums.append(nc.tensor.ldweights(dum_pool["prev"]))
                for kg in range(KT // KG):
                    pst = pp.tile([P, KG * P], bdt, tag="pst", bufs=2)
                    for q in range(KG):
                        k = kg * KG + q
                        t = nc.tensor.transpose(
                            pst[:, q * P : (q + 1) * P],
                            bftile[:, k * P : (k + 1) * P],
                            ident[:, :],
                        )
                        if q == 0:
                            for d in dums:
                                add_dep_helper(
                                    t.ins, d.ins, sync=False,
                                    reason="keep wait-absorber LDW before transposes",
                                )
                    put_evac(kg, pst[:, :].rearrange("p (k c) -> p k c", k=KG))
                dum_pool["prev"] = bftile[:, (KT - 1) * P : KT * P]

            def w_setup(j):
                c0 = j * P
                csz = min(P, C - c0)
                w_f32 = w_f32s[j]
                w_bf = wp.tile([P, F], bdt, tag="w_bf")
                if csz < P:
                    # pad rows feed the transpose below; keep them finite.
                    pad_base = (csz // 32) * 32
                    nc.vector.memset(w_bf[pad_base:P, :], 0.0)
                nc.vector.tensor_copy(w_bf[:csz, :], w_f32[:csz, :])

                def put(kg, src):
                    dst = wT[:, kg * KG : (kg + 1) * KG, c0 : c0 + P]
                    nc.scalar.activation(dst, src, AF.Identity)
                transpose_evac(w_bf, put)

                # ||w_c||^2 on the PE: diag of wT.T @ wT from the fp8
                # chunks just evacuated (pad cols are zero -> diag 0).
                pdg = pp.tile([P, 2 * 512], fdt, tag="ps")
                for d in range(KD):
                    nc.tensor.matmul(
                        pdg[:, 0:P],
                        wT[:, 2 * d : 2 * d + 2, c0 : c0 + P],
                        wT[:, 2 * d : 2 * d + 2, c0 : c0 + P],
                        start=(d == 0), stop=(d == KD - 1),
                        perf_mode=DR,
                    )
                masked = wp.tile([P, P], fdt, tag="wdiag", bufs=2)
                nc.vector.tensor_tensor(
                    masked[:, :], pdg[:, 0:P], ident[:, :], ALU.mult
                )
                w2col = wp.tile([P, 1], fdt, tag="w2col", bufs=2)
                nc.vector.tensor_reduce(
                    w2col[:csz, :], masked[:csz, :],
                    mybir.AxisListType.XYZW, ALU.add,
                )
                nc.gpsimd.dma_start(out=w2d[c0 : c0 + csz, :], in_=w2col[:csz, :])

            inv_sqrt_f = 1.0 / math.sqrt(F)

            def x_prep(i, x_f32, n_act_evacs=1):
                x_bf = xp.tile([P, F], bdt, tag="x_bf")
                nc.vector.tensor_copy(x_bf[:, :], x_f32[:, :])
                xsq = xp.tile([P, F], bdt, tag="xsq", bufs=2)
                x2c = xp.tile([P, 1], fdt, tag="x2c", bufs=2)
                # accum_out = sum_f (x/sqrt(F))^2 = ||x_b||^2 / F
                nc.scalar.activation(
                    xsq[:, :], x_f32[:, :], AF.Square,
                    scale=inv_sqrt_f, accum_out=x2c[:, :],
                )
                nc.vector.tensor_scalar_mul(negx2s[i][:, :], x2c[:, :], -1.0)

                def put(kg, src):
                    dst = xTs[i][:, kg * KG : (kg + 1) * KG, :]
                    if kg < n_act_evacs:
                        nc.scalar.activation(dst, src, AF.Identity)
                    else:
                        nc.vector.tensor_copy(dst, src)
                transpose_evac(x_bf, put)

            def gemm(i):
                b0 = i * P
                xT = xTs[i]
                ps = pp.tile([P, 2 * 512], fdt, tag="ps")
                # 9 DR matmuls per half; the first is the constant bias
                # rider contributing 1 * (-||w_c||^2/2) to every row.
                for n0, nsz in ((0, 512), (512, 488)):
                    nc.tensor.matmul(
                        ps[:, n0 : n0 + nsz],
                        xT1[:, 0:2, :],
                        wT[:, KT : KT + 2, n0 : n0 + nsz],
                        start=True, stop=False,
                        perf_mode=DR,
                    )
                    for d in range(KD):
                        nc.tensor.matmul(
                            ps[:, n0 : n0 + nsz],
                            xT[:, 2 * d : 2 * d + 2, :],
                            wT[:, 2 * d : 2 * d + 2, n0 : n0 + nsz],
                            start=False, stop=(d == KD - 1),
                            perf_mode=DR,
                        )

                o_sb = ep.tile([P, C], fdt, tag="o_sb")
                nc.scalar.activation(
                    o_sb[:, 0:C], ps[:, 0:C], AF.Identity,
                    bias=negx2s[i][:, 0:1], scale=2.0 / F,
                )
                nc.gpsimd.dma_start(out=out[b0 : b0 + P, :], in_=o_sb[:, :])

            # ---- Phase W: process w tiles as they land; x0..2 prep after --
            for j in range(CT):
                w_setup(j)
                if j < LAG:
                    x_prep(j, xfs.pop(j), n_act_evacs=0)
            # w2 gather (DRAM round-trip transposes the column to a row),
            # then drop -w2/2 into row 0 of the bias-rider wT plane.
            nc.gpsimd.dma_start(
                out=w2row[0:1, 0:C], in_=w2d[0:C, :].rearrange("c one -> one c")
            )
            nc.scalar.mul(
                wT[0:1, KT : KT + 1, 0:C].rearrange("p one c -> p (one c)"),
                w2row[0:1, 0:C], -0.5,
            )

            # ---- Phase G: 16 rounds of gemm(r) + x_prep(r+LAG) ----
            for r in range(BT):
                if r + LAG < BT:
                    xfs[r + LAG] = x_load(r + LAG)
                gemm(r)
                if r + LAG < BT:
                    x_prep(r + LAG, xfs.pop(r + LAG), n_act_evacs=1)

    return nc


def kernel(**inputs: np.ndarray) -> np.ndarray:
    global LAST_RESULTS
    x = np.ascontiguousarray(np.asarray(inputs["x"], dtype=np.float32))
    w = np.ascontiguousarray(np.asarray(inputs["w"], dtype=np.float32))
    assert x.shape == (B_TOTAL, F), x.shape
    assert w.shape == (C, F), w.shape

    from concourse.bass_utils import run_bass_kernel_spmd

    if "nc" not in _cache:
        nc = _build()
        _legalize_waits(nc)
        _cache["nc"] = nc
    nc = _cache["nc"]

    bs = B_TOTAL // N_CORES
    in_maps = [
        {"x": x[i * bs : (i + 1) * bs], "w": w} for i in range(N_CORES)
    ]
    res = run_bass_kernel_spmd(
        nc, in_maps, core_ids=list(range(N_CORES)),
        trace=bool(os.environ.get("BASS_TRACE")),
    )
    LAST_RESULTS = res
    return np.concatenate([r["out"] for r in res.results], axis=0)


if __name__ == "__main__":
    rng = np.random.default_rng(0)
    xs = rng.standard_normal((B_TOTAL, F), dtype=np.float32)
    ws = rng.standard_normal((C, F), dtype=np.float32) * math.sqrt(2.0 / F)
    o = kernel(x=xs, w=ws)
    print(o.shape, o.dtype, o[:2, :4])


# revision 9
# speedup vs baseline: 1.2099x; 1.2099x over previous
"""Trainium2 Bass kernel for nn_Euclidean (retrieval_knn).

Computes out[b, c] = -mean_f (x[b, f] - w[c, f])^2 for x [16384, 2048] f32,
w [1000, 2048] f32, via the algebraic expansion

    out = (2/F) * (x @ w.T  -  ||w_c||^2/2)  -  ||x_b||^2 / F

Sharding: data-parallel over the batch dim across 8 NeuronCores; w replicated.

Schedule (v4): lag-3 software pipeline; phase G round r issues
[x_load(r+3), gemm(r), x_prep(r+3)] so the PE stays HAM-warm.

  - w loads split across BOTH HWDGE rings (w0-3 scalar, w4-7 sync); x
    loads queue behind w4-7 on the sync ring FIFO so w gets full DMA
    bandwidth first.  All issues precede any compute instruction.
  - ||w_c||^2 comes from the PE: 8 DoubleRow matmuls square-contract the
    already-transposed fp8 wT chunks (diag of wT.T@wT in an idle gemm
    PSUM buffer), then DVE identity-mask multiply + ScalarE accumulate
    extract the diagonal.  This keeps the 2us-per-tile Square
    activations off the front-critical ScalarE queue.
  - w2 column -> row via DRAM round-trip on the GpSimd SWDGE ring
    (never blocks HWDGE x loads); -w2/2 lands in row 0 of the bias-rider
    wT plane.
  - gemm output stores ride GpSimd SWDGE too.

Per-x-tile prep: HWDGE DMA fp32 -> DVE bf16 cast -> ScalarE Square
activation with fp32 accum giving ||x_b||^2/F (ScalarE has slack in
phase G) -> TensorE bf16 transposes (2 groups of 8, one [128,1024] PSUM
tile each) -> 1 ScalarE + 1 DVE evac -> fp8e4 SBUF.

GEMM: 9 DoubleRow fp8 matmuls per 512-col half accumulate into one
[128,1024] PSUM tile (two banks).  The first plane-pair is a constant:
row 0 of its xT plane is all-ones and row 0 of its wT plane holds
-||w_c||^2/2 (fp8), so the bias term rides the same DR pipeline with no
weight-mode switch.  lhsT/rhs are 3D APs [128 ki, 2 plane, free] whose
planes are adjacent 128-k chunks (plane stride %16==0 satisfies
s3_lw_dual_fp8_restrictions).  One ScalarE Identity evacuates both PSUM
banks with scale=2/F and per-partition bias -||x_b||^2/F.

Walrus encodes at most one semaphore wait per LDWEIGHTS/MM struct: dummy
standalone LDWEIGHTS absorb cross-engine waits ahead of each transpose
group (add_dep_helper keeps them ordered), and _legalize_waits splits any
remaining multi-wait instructions.
"""

import math
import os
import sys

import numpy as np

if "/opt/trn_rl_repo" not in sys.path:
    sys.path.insert(0, "/opt/trn_rl_repo")

N_CORES = 8
B_TOTAL = 16384
F = 2048
C = 1000

_cache = {}
LAST_RESULTS = None


def _legalize_waits(nc):
    """Walrus encodes at most ONE sync-wait per instruction struct, but Tile's
    sem assignment freely attaches several. Split: hoist all but the last wait
    onto standalone EventSemaphore instructions (pure sem-op carriers) placed
    immediately before the over-limit instruction on the same engine queue."""
    import bass_rust
    import concourse.mybir as mybir

    n = 0
    for f in nc.m.functions:
        for bb in f.blocks:
            newlist = []
            for inst in bb.instructions:
                si = inst.sync_info
                if si is not None and len(si.on_wait) > 1:
                    waits = list(si.on_wait)
                    for w in waits[:-1]:
                        ev = mybir.InstEventSemaphore(
                            name=f"waitsplit_{n}", ins=[], outs=[]
                        )
                        ev.engine = inst.engine
                        ev.sync_info = bass_rust.SyncInfo(on_wait=[w], on_update=[])
                        newlist.append(ev)
                        n += 1
                    inst.sync_info = bass_rust.SyncInfo(
                        on_wait=[waits[-1]], on_update=list(si.on_update)
                    )
                newlist.append(inst)
            bb.instructions = newlist
    return n


def _build():
    import concourse.bass as bass
    import concourse.mybir as mybir
    from bass_rust import add_dep_helper
    from concourse.masks import make_identity
    from concourse.tile import TileContext

    P = 128
    KT = F // P                 # 16 contraction chunks of 128
    KD = KT // 2                # 8 DoubleRow plane-pairs of 256
    B = B_TOTAL // N_CORES      # 2048 batch rows per core
    BT = B // P                 # 16 batch chunks
    CP = 1024                   # padded class dim
    CT = CP // P                # 8 class chunks
    KG = 8                      # k-chunks per PSUM transpose group
    LAG = 3                     # x tiles prepped ahead of their gemm
    f8 = mybir.dt.float8e4
    bdt = mybir.dt.bfloat16
    fdt = mybir.dt.float32
    AF = mybir.ActivationFunctionType
    ALU = mybir.AluOpType
    DR = mybir.MatmulPerfMode.DoubleRow

    nc = bass.Bass()
    x = nc.dram_tensor("x", [B, F], fdt, kind="ExternalInput")
    w = nc.dram_tensor("w", [C, F], fdt, kind="ExternalInput")
    out = nc.dram_tensor("out", [B, C], fdt, kind="ExternalOutput")

    with TileContext(nc) as tc:
        with (
            tc.tile_pool(name="consts", bufs=1) as constp,
            tc.tile_pool(name="wstage", bufs=3) as wp,
            tc.tile_pool(name="xstage", bufs=3) as xp,
            tc.tile_pool(name="evac", bufs=3) as ep,
            tc.tile_pool(name="dram", bufs=1, space="DRAM") as dp,
            tc.tile_pool(name="psum", bufs=2, space="PSUM") as pp,
        ):
            # ---- DMA issues first: w on both rings, x behind w on sync ----
            w_f32s = []
            for j in range(CT):
                c0 = j * P
                csz = min(P, C - c0)
                w_f32 = wp.tile([P, F], fdt, tag="w_f32", bufs=8,
                                name=f"w_f32_{j}")
                eng = nc.scalar if j < CT // 2 else nc.sync
                eng.dma_start(out=w_f32[:csz, :], in_=w[c0 : c0 + csz, :])
                w_f32s.append(w_f32)

            def x_load(i):
                x_f32 = xp.tile([P, F], fdt, tag="x_f32", bufs=4,
                                name=f"x_f32_{i}")
                nc.sync.dma_start(out=x_f32[:, :], in_=x[i * P : (i + 1) * P, :])
                return x_f32

            xfs = {}
            for j in range(LAG):
                xfs[j] = x_load(j)

            # ---- constants (emitted after the issues; run during the DMA) --
            ident = constp.tile([P, P], bdt)
            make_identity(nc, ident[:, :])
            pwarm = pp.tile([P, P], bdt, tag="pst", bufs=2)
            nc.tensor.transpose(pwarm[:, :], ident[:, :], ident[:, :])

            # preload both ACT table sets (Square + Identity) off-path
            tw0 = wp.tile([1, 1], fdt, tag="actwarm0")
            tw1 = wp.tile([1, 1], fdt, tag="actwarm1")
            nc.vector.memset(tw0[:, :], 0.0)
            nc.scalar.activation(tw1[:, :], tw0[:, :], AF.Square)
            nc.scalar.activation(tw1[:, :], tw0[:, :], AF.Identity)

            # wT planes 0..15 hold w^T fp8; plane pair 16/17 is the bias
            # rider: row 0 of plane 16 carries -||w_c||^2/2, rest zero.
            wT = constp.tile([P, KT + 2, CP], f8)
            xT1 = constp.tile([P, 2, P], f8)      # ones@row0 bias-rider lhsT
            nc.vector.memset(wT[:, KT : KT + 2, :], 0.0)
            nc.vector.memset(xT1[:, :, :], 0.0)
            nc.vector.memset(xT1[0:1, 0:1, :], 1.0)
            w2row = constp.tile([1, CP], fdt)
            w2d = dp.tile([CP, 1], fdt)
            xTs = [
                constp.tile([P, KT, P], f8, name=f"xT_{i}") for i in range(BT)
            ]
            negx2s = [
                constp.tile([P, 1], fdt, name=f"negx2_{i}") for i in range(BT)
            ]

            dum_pool = {"prev": None}

            def transpose_evac(bftile, put_evac):
                """16 bf16 transposes in 2 [128,1024] PSUM groups + evacs."""
                dums = [nc.tensor.ldweights(bftile[:, 0:P])]
                if dum_pool["prev"] is not None:
                    dums.append(nc.tensor.ldweights(dum_pool["prev"]))
                for kg in range(KT // KG):
                    pst = pp.tile([P, KG * P], bdt, tag="pst", bufs=2)
                    for q in range(KG):
                        k = kg * KG + q
                        t = nc.tensor.transpose(
                            pst[:, q * P : (q + 1) * P],
                            bftile[:, k * P : (k + 1) * P],
                            ident[:, :],
                        )
                        if q == 0:
                            for d in dums:
                                add_dep_helper(
                                    t.ins, d.ins, sync=False,
                                    reason="keep wait-absorber LDW before transposes",
                                )
                    put_evac(kg, pst[:, :].rearrange("p (k c) -> p k c", k=KG))
                dum_pool["prev"] = bftile[:, (KT - 1) * P : KT * P]

            def w_setup(j):
                c0 = j * P
                csz = min(P, C - c0)
                w_f32 = w_f32s[j]
                # norm first: it gates the bias-rider fill and reads
                # w_f32 directly, so it never waits on the cast chain.
                wsq = wp.tile([P, F], bdt, tag="wsq", bufs=2)
                w2col = wp.tile([P, 1], fdt, tag="w2col", bufs=2)
                nc.scalar.activation(
                    wsq[:csz, :], w_f32[:csz, :], AF.Square,
                    accum_out=w2col[:csz, :],
                )
                nc.gpsimd.dma_start(out=w2d[c0 : c0 + csz, :], in_=w2col[:csz, :])

                w_bf = wp.tile([P, F], bdt, tag="w_bf")
                if csz < P:
                    # pad rows feed the transpose below; keep them finite.
                    pad_base = (csz // 32) * 32
                    nc.vector.memset(w_bf[pad_base:P, :], 0.0)
                nc.vector.tensor_copy(w_bf[:csz, :], w_f32[:csz, :])

                def put(kg, src):
                    dst = wT[:, kg * KG : (kg + 1) * KG, c0 : c0 + P]
                    # ScalarE also carries the 2us Square per tile, so on
                    # even tiles give it only one evac; DVE takes the rest.
                    if kg == 0 and j % 2 == 0:
                        nc.scalar.activation(dst, src, AF.Identity)
                    else:
                        nc.vector.tensor_copy(dst, src)
                transpose_evac(w_bf, put)

            inv_sqrt_f = 1.0 / math.sqrt(F)

            def x_prep(i, x_f32, n_act_evacs=1):
                x_bf = xp.tile([P, F], bdt, tag="x_bf")
                nc.vector.tensor_copy(x_bf[:, :], x_f32[:, :])
                xsq = xp.tile([P, F], bdt, tag="xsq", bufs=2)
                x2c = xp.tile([P, 1], fdt, tag="x2c", bufs=2)
                # accum_out = sum_f (x/sqrt(F))^2 = ||x_b||^2 / F
                nc.scalar.activation(
                    xsq[:, :], x_f32[:, :], AF.Square,
                    scale=inv_sqrt_f, accum_out=x2c[:, :],
                )
                nc.vector.tensor_scalar_mul(negx2s[i][:, :], x2c[:, :], -1.0)

                def put(kg, src):
                    dst = xTs[i][:, kg * KG : (kg + 1) * KG, :]
                    if kg < n_act_evacs:
                        nc.scalar.activation(dst, src, AF.Identity)
                    else:
                        nc.vector.tensor_copy(dst, src)
                transpose_evac(x_bf, put)

            def gemm(i):
                b0 = i * P
                xT = xTs[i]
                ps = pp.tile([P, 2 * 512], fdt, tag="ps")
                # 9 DR matmuls per half; the first is the constant bias
                # rider contributing 1 * (-||w_c||^2/2) to every row.
                for n0, nsz in ((0, 512), (512, 488)):
                    nc.tensor.matmul(
                        ps[:, n0 : n0 + nsz],
                        xT1[:, 0:2, :],
                        wT[:, KT : KT + 2, n0 : n0 + nsz],
                        start=True, stop=False,
                        perf_mode=DR,
                    )
                    for d in range(KD):
                        nc.tensor.matmul(
                            ps[:, n0 : n0 + nsz],
                            xT[:, 2 * d : 2 * d + 2, :],
                            wT[:, 2 * d : 2 * d + 2, n0 : n0 + nsz],
                            start=False, stop=(d == KD - 1),
                            perf_mode=DR,
                        )

                o_sb = ep.tile([P, C], fdt, tag="o_sb")
                nc.scalar.activation(
                    o_sb[:, 0:C], ps[:, 0:C], AF.Identity,
                    bias=negx2s[i][:, 0:1], scale=2.0 / F,
                )
                nc.gpsimd.dma_start(out=out[b0 : b0 + P, :], in_=o_sb[:, :])

            # ---- Phase W: process w tiles as they land; x0..2 prep after --
            for j in range(CT):
                w_setup(j)
                if j < LAG:
                    x_prep(j, xfs.pop(j), n_act_evacs=0)
            # w2 gather (DRAM round-trip transposes the column to a row),
            # then drop -w2/2 into row 0 of the bias-rider wT plane.
            nc.gpsimd.dma_start(
                out=w2row[0:1, 0:C], in_=w2d[0:C, :].rearrange("c one -> one c")
            )
            nc.scalar.mul(
                wT[0:1, KT : KT + 1, 0:C].rearrange("p one c -> p (one c)"),
                w2row[0:1, 0:C], -0.5,
            )

            # ---- Phase G: 16 rounds of gemm(r) + x_prep(r+LAG) ----
            for r in range(BT):
                if r + LAG < BT:
                    xfs[r + LAG] = x_load(r + LAG)
                gemm(r)
                if r + LAG < BT:
                    x_prep(r + LAG, xfs.pop(r + LAG), n_act_evacs=1)

    return nc


def kernel(**inputs: np.ndarray) -> np.ndarray:
    global LAST_RESULTS
    x = np.ascontiguousarray(np.asarray(inputs["x"], dtype=np.float32))
    w = np.ascontiguousarray(np.asarray(inputs["w"], dtype=np.float32))
    assert x.shape == (B_TOTAL, F), x.shape
    assert w.shape == (C, F), w.shape

    from concourse.bass_utils import run_bass_kernel_spmd

    if "nc" not in _cache:
        nc = _build()
        _legalize_waits(nc)
        _cache["nc"] = nc
    nc = _cache["nc"]

    bs = B_TOTAL // N_CORES
    in_maps = [
        {"x": x[i * bs : (i + 1) * bs], "w": w} for i in range(N_CORES)
    ]
    res = run_bass_kernel_spmd(
        nc, in_maps, core_ids=list(range(N_CORES)),
        trace=bool(os.environ.get("BASS_TRACE")),
    )
    LAST_RESULTS = res
    return np.concatenate([r["out"] for r in res.results], axis=0)


if __name__ == "__main__":
    rng = np.random.default_rng(0)
    xs = rng.standard_normal((B_TOTAL, F), dtype=np.float32)
    ws = rng.standard_normal((C, F), dtype=np.float32) * math.sqrt(2.0 / F)
    o = kernel(x=xs, w=ws)
    print(o.shape, o.dtype, o[:2, :4])
